# revision 9
# baseline (speedup 1.0000x reference)
"""Trainium2 Bass kernel for nn_LinearSelfAttention (local windowed + global linear attention).

Self-contained: takes FULL inputs as numpy arrays, shards across 8 NeuronCores
(batch x head-group), runs a Bass/Tile kernel per core, and combines partial
outputs on the host.

Sharding: core c -> (batch bi = c // 4, head group g = c % 4).
Each core handles 2 local heads {2g, 2g+1} and 2 global heads {8+2g, 8+2g+1},
computes its QKV projections, attention, and a partial output
attn_slab @ Wo[slab_rows]; the host sums the 4 partials per batch and adds bo.
"""

import numpy as np
import ml_dtypes

B = 2
T = 4096
D = 1024
DH = 64
W = 128
NW = T // W            # 32 windows
KS = D // 128          # 8 contraction slices
CH = 1024              # projection T-chunk
NCH = T // CH          # 4
NEG = float(np.finfo(np.float32).min)  # -3.4028235e38
N_CORES = 8

BF16 = ml_dtypes.bfloat16

_CACHE = {}


def _build_nc(phases=("proj", "local", "glob", "wo")):
    import concourse.bacc as bacc
    import concourse.bass as bass
    import concourse.mybir as mybir
    import concourse.tile as tile

    F32 = mybir.dt.float32
    BF = mybir.dt.bfloat16
    ts = bass.ts
    AX = mybir.AxisListType.X
    Exp = mybir.ActivationFunctionType.Exp
    add = mybir.AluOpType.add
    mult = mybir.AluOpType.mult
    amax = mybir.AluOpType.max

    nc = bacc.Bacc("TRN2", target_bir_lowering=False, debug=False,
                   num_devices=N_CORES)

    xT = nc.dram_tensor("xT", [D, T], BF, kind="ExternalInput").ap()
    wq = nc.dram_tensor("wq", [D, 256], BF, kind="ExternalInput").ap()
    wk = nc.dram_tensor("wk", [D, 256], BF, kind="ExternalInput").ap()
    wv = nc.dram_tensor("wv", [D, 256], BF, kind="ExternalInput").ap()
    wo = nc.dram_tensor("wo", [256, D], BF, kind="ExternalInput").ap()
    colmask = nc.dram_tensor("colmask", [1, T + 256], BF, kind="ExternalInput").ap()
    rowmask = nc.dram_tensor("rowmask", [128, NW], F32, kind="ExternalInput").ap()
    vmask = nc.dram_tensor("vmask", [128, NW], F32, kind="ExternalInput").ap()
    kmask = nc.dram_tensor("kmask", [1, T], BF, kind="ExternalInput").ap()
    out = nc.dram_tensor("out", [T, D], F32, kind="ExternalOutput").ap()

    from concourse.masks import make_identity

    with tile.TileContext(nc) as tc:
        with (
            tc.tile_pool(name="const", bufs=1) as const,
            tc.tile_pool(name="persist", bufs=1) as persist,
            tc.tile_pool(name="xp", bufs=2) as xp,
            tc.tile_pool(name="work", bufs=3) as work,
            tc.tile_pool(name="small", bufs=4) as small,
            tc.tile_pool(name="proj_ps", bufs=2, space="PSUM") as proj_ps,
            tc.tile_pool(name="dots_ps", bufs=2, space="PSUM") as dots_ps,
            tc.tile_pool(name="trans_ps", bufs=2, space="PSUM") as trans_ps,
            tc.tile_pool(name="pv_ps", bufs=2, space="PSUM") as pv_ps,
        ):
            # ---- constants ----
            ident = const.tile([128, 128], BF, tag="ident")
            make_identity(nc, ident[:])
            rm_sb = const.tile([128, NW], F32, tag="rm")
            nc.sync.dma_start(rm_sb[:], rowmask[:])
            vm_sb = const.tile([128, NW], F32, tag="vm")
            nc.sync.dma_start(vm_sb[:], vmask[:])
            km_sb = const.tile([1, T], BF, tag="km")
            nc.sync.dma_start(km_sb[:], kmask[:])
            ones1 = const.tile([1, 128], BF, tag="ones1")
            nc.vector.memset(ones1[:], 1.0)

            wq_sb = const.tile([128, KS, 256], BF, tag="wq")
            wk_sb = const.tile([128, KS, 256], BF, tag="wk")
            wv_sb = const.tile([128, KS, 256], BF, tag="wv")
            for k in range(KS):
                nc.sync.dma_start(wq_sb[:, k, :], wq[ts(k, 128), :])
                nc.sync.dma_start(wk_sb[:, k, :], wk[ts(k, 128), :])
                nc.sync.dma_start(wv_sb[:, k, :], wv[ts(k, 128), :])
            wo_sb = const.tile([128, 2, D], BF, tag="wo")
            for s in range(2):
                nc.sync.dma_start(wo_sb[:, s, :], wo[ts(s, 128), :])

            # ---- persistent activations ----
            qTa = [persist.tile([65, T], BF, tag=f"qTa{h}", name=f"qTa{h}")
                   for h in range(2)]
            kTa = [persist.tile([65, T + 256], BF, tag=f"kTa{h}", name=f"kTa{h}")
                   for h in range(2)]
            v_all = persist.tile([128, NW, 4, DH], BF, tag="v_all")
            expq = persist.tile([128, NW, 2, DH], BF, tag="expq")
            expk = persist.tile([128, T], BF, tag="expk")
            kn_nat = persist.tile([128, NW, 2, DH], BF, tag="kn_nat")
            qnT = persist.tile([128, T], BF, tag="qnT")
            attnT = persist.tile([128, 2, T], BF, tag="attnT")
            sk = persist.tile([128, 8], F32, tag="sk")
            sq = persist.tile([128, NW, 2], F32, tag="sq")

            for h in range(2):
                nc.vector.memset(qTa[h][64:65, :], 1.0)
                nc.sync.dma_start(kTa[h][64:65, :], colmask[:])
                nc.vector.memset(kTa[h][0:64, 0:128], 0.0)
                nc.vector.memset(kTa[h][0:64, T + 128:T + 256], 0.0)

            # ---- projections, chunked over T ----
            for c in range(NCH):
                xt = xp.tile([128, KS, CH], BF, tag="xt")
                for k in range(KS):
                    nc.sync.dma_start(xt[:, k, :], xT[ts(k, 128), ts(c, CH)])
                for nsub in range(CH // 512):
                    tcol = c * CH + nsub * 512
                    ridx = 2 * c + nsub
                    # local qT (heads l0, l1), scaled by d^-0.5
                    ps = proj_ps.tile([128, 512], F32, tag="proj", name="ps_q")
                    for k in range(KS):
                        nc.tensor.matmul(ps[:], wq_sb[:, k, 0:128],
                                         xt[:, k, ts(nsub, 512)],
                                         start=(k == 0), stop=(k == KS - 1))
                    nc.scalar.mul(qTa[0][0:64, tcol:tcol + 512], ps[0:64, :], 0.125)
                    nc.scalar.mul(qTa[1][0:64, tcol:tcol + 512], ps[64:128, :], 0.125)
                    # local kT
                    ps = proj_ps.tile([128, 512], F32, tag="proj", name="ps_k")
                    for k in range(KS):
                        nc.tensor.matmul(ps[:], wk_sb[:, k, 0:128],
                                         xt[:, k, ts(nsub, 512)],
                                         start=(k == 0), stop=(k == KS - 1))
                    nc.scalar.copy(kTa[0][0:64, 128 + tcol:128 + tcol + 512],
                                   ps[0:64, :])
                    nc.scalar.copy(kTa[1][0:64, 128 + tcol:128 + tcol + 512],
                                   ps[64:128, :])
                    # global kT with additive NEG mask via K=1 aug matmul
                    ps = proj_ps.tile([128, 512], F32, tag="proj", name="ps_kg")
                    for k in range(KS):
                        nc.tensor.matmul(ps[:], wk_sb[:, k, 128:256],
                                         xt[:, k, ts(nsub, 512)],
                                         start=(k == 0), stop=False)
                    nc.tensor.matmul(ps[:], ones1[:],
                                     km_sb[:, tcol:tcol + 512],
                                     start=False, stop=True)
                    nc.scalar.activation(expk[0:64, tcol:tcol + 512], ps[0:64, :],
                                         Exp, accum_out=sk[0:64, ridx:ridx + 1])
                    nc.scalar.activation(expk[64:128, tcol:tcol + 512],
                                         ps[64:128, :], Exp,
                                         accum_out=sk[64:128, ridx:ridx + 1])
                for tt in range(CH // 128):
                    w = c * (CH // 128) + tt
                    # v for all 4 heads (natural layout); global half masked
                    ps = proj_ps.tile([128, 256], F32, tag="proj", name="ps_v")
                    for k in range(KS):
                        nc.tensor.matmul(ps[:], xt[:, k, ts(tt, 128)],
                                         wv_sb[:, k, :],
                                         start=(k == 0), stop=(k == KS - 1))
                    nc.scalar.copy(v_all[:, w, 0:2, :], ps[:, 0:128])
                    nc.vector.tensor_scalar(out=v_all[:, w, 2:4, :],
                                            in0=ps[:, 128:256],
                                            scalar1=vm_sb[:, w:w + 1],
                                            scalar2=None, op0=mult)
                    # global q natural -> exp (softmax max-sub skipped; |q| small)
                    ps = proj_ps.tile([128, 128], F32, tag="proj", name="ps_qg")
                    for k in range(KS):
                        nc.tensor.matmul(ps[:], xt[:, k, ts(tt, 128)],
                                         wq_sb[:, k, 128:256],
                                         start=(k == 0), stop=(k == KS - 1))
                    nc.scalar.activation(expq[:, w, :, :], ps[:], Exp)

            # ---- local windowed attention ----
            if "local" not in phases:
                nc.vector.memset(attnT[:, 0, :], 0.0)
            for w in (range(NW) if "local" in phases else ()):
                jlist = [j for j in (0, 1, 2) if 0 <= w - 1 + j < NW]
                j0, j1 = jlist[0], jlist[-1]
                pv = pv_ps.tile([128, 128], F32, tag="pv", name="pv")
                for h in range(2):
                    dots = dots_ps.tile([128, 384], F32, tag="dots", name="dots")
                    nc.tensor.matmul(dots[:], qTa[h][:, ts(w, 128)],
                                     kTa[h][:, 128 * w:128 * w + 384],
                                     start=True, stop=True)
                    msk = work.tile([128, 384], F32, tag="msk", name="msk")
                    nc.vector.tensor_scalar(out=msk[:], in0=dots[:],
                                            scalar1=rm_sb[:, w:w + 1],
                                            scalar2=NEG, op0=add, op1=amax)
                    nmx = small.tile([128, 1], F32, tag="nmx", name="nmx")
                    nc.vector.reduce_max(nmx[:], msk[:], axis=AX, negate=True)
                    pex = work.tile([128, 384], F32, tag="pex", name="pex")
                    ssum = small.tile([128, 1], F32, tag="ssum", name="ssum")
                    nc.scalar.activation(pex[:], msk[:], Exp, bias=nmx[:],
                                         accum_out=ssum[:])
                    rec = small.tile([128, 1], F32, tag="rec", name="rec")
                    nc.vector.reciprocal(rec[:], ssum[:])
                    pbf = work.tile([128, 384], BF, tag="pbf", name="pbf")
                    nc.vector.tensor_scalar(out=pbf[:], in0=pex[:],
                                            scalar1=rec[:], scalar2=None,
                                            op0=mult)
                    ptp = trans_ps.tile([128, 384], F32, tag="trans", name="ptp")
                    for j in jlist:
                        nc.tensor.matmul(ptp[:, ts(j, 128)], pbf[:, ts(j, 128)],
                                         ident[:], start=True, stop=True)
                    ptb = work.tile([128, 384], BF, tag="ptb", name="ptb")
                    nc.scalar.copy(ptb[:, 128 * j0:128 * (j1 + 1)],
                                   ptp[:, 128 * j0:128 * (j1 + 1)])
                    for i, j in enumerate(jlist):
                        nc.tensor.matmul(pv[64 * h:64 * h + 64, :],
                                         v_all[:, w - 1 + j, h, :],
                                         ptb[:, ts(j, 128)],
                                         start=(i == 0), stop=(i == len(jlist) - 1))
                nc.scalar.copy(attnT[:, 0, ts(w, 128)], pv[:])

            # ---- global linear attention ----
            if "glob" not in phases:
                nc.vector.memset(attnT[:, 1, :], 0.0)
            # normalize k: rk = 1/sum_t exp(k); diag(rk) folded into transposes
            glob_iter = lambda it, key="glob": it if (key in phases or "glob" in phases) else ()
            skt = small.tile([128, 1], F32, tag="skt", name="skt")
            nc.vector.reduce_sum(skt[:], sk[:], axis=AX)
            rk = small.tile([128, 1], F32, tag="rk", name="rk")
            nc.vector.reciprocal(rk[:], skt[:])
            diagk = persist.tile([128, 128], BF, tag="diagk")
            nc.vector.tensor_scalar(out=diagk[:], in0=ident[:],
                                    scalar1=rk[:], scalar2=None, op0=mult)
            for w in glob_iter(range(NW), "globk"):
                knp = trans_ps.tile([128, 128], F32, tag="trans", name="knp")
                nc.tensor.matmul(knp[:], expk[:, ts(w, 128)], diagk[:],
                                 start=True, stop=True)
                nc.scalar.copy(kn_nat[:, w, :, :], knp[:])

            # normalize q: rq = 0.125/sum_d exp(q); diag(rq) folded into transposes
            nc.vector.reduce_sum(sq[:], expq[:], axis=AX)
            rq = persist.tile([128, NW, 2], F32, tag="rq")
            nc.vector.reciprocal(rq[:], sq[:])
            rq8 = persist.tile([128, NW, 2], F32, tag="rq8")
            nc.vector.tensor_scalar(out=rq8[:], in0=rq[:], scalar1=0.125,
                                    scalar2=None, op0=mult)
            for w in glob_iter(range(NW), "globq"):
                qnp = trans_ps.tile([128, 128], F32, tag="trans", name="qnp")
                for h in range(2):
                    dq = work.tile([128, 128], BF, tag="dq", name="dq")
                    nc.vector.tensor_scalar(out=dq[:], in0=ident[:],
                                            scalar1=rq8[:, w, h:h + 1],
                                            scalar2=None, op0=mult)
                    nc.tensor.matmul(qnp[64 * h:64 * h + 64, :],
                                     expq[:, w, h, :], dq[:],
                                     start=True, stop=True)
                nc.scalar.copy(qnT[:, ts(w, 128)], qnp[:])

            # context: ctxT[e,d] = sum_t v[t,e] kn[t,d]; then transpose -> ctx[d,e]
            ctx_bf = persist.tile([128, 64], BF, tag="ctx_bf")
            for h in glob_iter(range(2), "globc"):
                pr = slice(64 * h, 64 * h + 64)
                cps = trans_ps.tile([128, 64], F32, tag="trans", name="cps")
                for w in range(NW):
                    nc.tensor.matmul(cps[pr, :], v_all[:, w, 2 + h, :],
                                     kn_nat[:, w, h, :],
                                     start=(w == 0), stop=(w == NW - 1))
                ctxT_bf = work.tile([128, 64], BF, tag="ctxT", name="ctxT")
                nc.scalar.copy(ctxT_bf[pr, :], cps[pr, :])
                cps2 = trans_ps.tile([128, 64], F32, tag="trans", name="cps2")
                nc.tensor.matmul(cps2[pr, :], ctxT_bf[pr, :],
                                 ident[pr, 64 * h:64 * h + 64],
                                 start=True, stop=True)
                nc.scalar.copy(ctx_bf[pr, :], cps2[pr, :])

            # outgT[e,t] = ctx[d,e].T @ qnT[d,t]
            for n in glob_iter(range(T // 512), "globc"):
                og = trans_ps.tile([128, 512], F32, tag="trans", name="og")
                for h in range(2):
                    pr = slice(64 * h, 64 * h + 64)
                    nc.tensor.matmul(og[pr, :], ctx_bf[pr, :],
                                     qnT[pr, ts(n, 512)], start=True, stop=True)
                nc.scalar.copy(attnT[:, 1, ts(n, 512)], og[:])

            # ---- output projection: out[t,:] += attnT.T @ wo ----
            for w in (range(NW) if "wo" in phases else ()):
                for nh in range(2):
                    ops_ = proj_ps.tile([128, 512], F32, tag="proj", name="ps_o")
                    for s in range(2):
                        nc.tensor.matmul(ops_[:], attnT[:, s, ts(w, 128)],
                                         wo_sb[:, s, ts(nh, 512)],
                                         start=(s == 0), stop=(s == 1))
                    ost = work.tile([128, 512], F32, tag="ost", name="ost")
                    if nh == 0:
                        nc.scalar.copy(ost[:], ops_[:])
                    else:
                        nc.vector.tensor_copy(ost[:], ops_[:])
                    nc.sync.dma_start(out[ts(w, 128), ts(nh, 512)], ost[:])

    nc.compile()
    return nc


def get_nc(phases=("proj", "local", "glob", "wo")):
    key = tuple(sorted(phases))
    if key not in _CACHE:
        _CACHE[key] = _build_nc(phases)
    return _CACHE[key]


def make_in_maps(x, input_mask, Wq, Wk, Wv, Wo):
    x = np.asarray(x, dtype=np.float32)
    input_mask = np.asarray(input_mask)
    valid = ~input_mask  # (B, T) True = keep
    Wq = np.asarray(Wq, dtype=np.float32)
    Wk = np.asarray(Wk, dtype=np.float32)
    Wv = np.asarray(Wv, dtype=np.float32)
    Wo = np.asarray(Wo, dtype=np.float32)

    xT_b = [np.ascontiguousarray(x[bi].T).astype(BF16) for bi in range(B)]

    in_maps = []
    for c in range(N_CORES):
        bi, g = divmod(c, 4)
        cols = np.r_[64 * 2 * g:64 * (2 * g + 2),
                     64 * (8 + 2 * g):64 * (8 + 2 * g + 2)]
        vm = valid[bi]  # (T,)
        mNEG = np.where(vm, np.float32(0.0), np.float32(NEG))
        colmask = np.full((1, T + 256), NEG, np.float32)
        colmask[0, 128:128 + T] = mNEG
        rowmask = mNEG.reshape(NW, W).T.copy()  # (128, NW)
        vmask = vm.reshape(NW, W).T.astype(np.float32)
        kmask = mNEG.reshape(1, T)
        in_maps.append({
            "xT": xT_b[bi],
            "wq": np.ascontiguousarray(Wq[:, cols]).astype(BF16),
            "wk": np.ascontiguousarray(Wk[:, cols]).astype(BF16),
            "wv": np.ascontiguousarray(Wv[:, cols]).astype(BF16),
            "wo": np.ascontiguousarray(Wo[cols, :]).astype(BF16),
            "colmask": colmask.astype(BF16),
            "rowmask": np.ascontiguousarray(rowmask),
            "vmask": np.ascontiguousarray(vmask),
            "kmask": kmask.astype(BF16),
        })
    return in_maps


def combine_outputs(results, bo):
    out = np.zeros((B, T, D), np.float32)
    for c in range(N_CORES):
        out[c // 4] += results[c]["out"]
    out += np.asarray(bo, dtype=np.float32)
    return out


def kernel(x, input_mask, Wq, Wk, Wv, Wo, bo):
    from concourse.bass_utils import run_bass_kernel_spmd
    nc = get_nc()
    in_maps = make_in_maps(x, input_mask, Wq, Wk, Wv, Wo)
    res = run_bass_kernel_spmd(nc, in_maps, core_ids=list(range(N_CORES)))
    return combine_outputs(res.results, bo)
